# revision 75
# baseline (speedup 1.0000x reference)
"""Trainium2 Bass kernel for nn_Attention_13718125543518.

Dense MHA (B=16, N=1024, DIM=512, H=8, DH=64) with additive positional
bias and softmax:  y = softmax(q k^T / sqrt(dh) + pos_bias) v @ Wo^T.

Sharding: data-parallel over batch. Each of the 8 cores processes 2
batches and all 8 heads; no cross-core communication.

Device-side algorithm (per core, all matmul operands fp16, PSUM fp32):
  - host precomputes xT[c,i] (x transposed), EB[h,j,i] = exp(pos_bias[h,i,j]),
    and weight layouts; the 1/sqrt(dh) scale is folded into Wq.
  - qkT_h[:,i] = [Wq_h^T | Wk_h^T]^T . xT  -> [128, 2048] (rows 0:64 = q^T,
    rows 64:128 = k^T), plus a partition-swapped copy kqT (DMA) so both the
    (0,0) and (64,0) PE row-tiles can compute score tiles concurrently.
  - scores s^T[j,i] = sum_d k[j,d] q[i,d] (K=64 matmuls, two per PE pass via
    row tiling), ACT evacuates PSUM with exp(), DVE multiplies by EB
    (factorized softmax: exp(qk+b) = exp(qk)*exp(b); logits are O(6) so no
    max-subtraction is needed).
  - AV: out^T[d,i] = sum_j v'[j,d] P[j,i] with v' = [v_h | 1] (M=65): row 64
    accumulates the softmax denominator Z[i] for free.
  - normalize (pipelined two steps behind AV): spread Z across partitions by
    DMA, exact DVE reciprocal on [128, n/128], broadcast to 64 partitions
    with doubling DMAs on the gpsimd queue, multiply.
  - y[i,f] = sum_h out_h^T . Wo_h^T with head pairs repacked to K=128.
"""

import threading
from contextlib import ExitStack

import ml_dtypes
import numpy as np

import concourse.bacc as bacc
import concourse.bass as bass
import concourse.mybir as mybir
import concourse.tile as tile
from concourse.bass_utils import run_bass_kernel_spmd

B, N, DIM, H, DH = 16, 1024, 512, 8, 64
SCALE = DH**-0.5
NCORES = 8
NB = B // NCORES  # batches per core
F16 = mybir.dt.float16
F32 = mybir.dt.float32
F8 = mybir.dt.float8e4
E4 = ml_dtypes.float8_e4m3
DR = mybir.MatmulPerfMode.DoubleRow
# q/k weights are pre-scaled by 2^5 into e4m3's normal range (their raw
# sigma 0.02 sits in the subnormals); the 1/WS^2 and the 1/sqrt(dh) of the
# attention logits are folded into the exp's scale parameter.
WS = 32.0

_lock = threading.Lock()
_built = {}


def emit(tc, xt, xt8, eb, wqk8, wkq08, wvt, wot, y, nb=NB, h_=H, n=N, dim=DIM):
    """Emit the per-core program. xt:[dim,nb*n] f16; xt8:[2,128,2*nb*n] fp8
    (DoubleRow layout: channel c = cc2*256 + slot*128 + p); eb:[h,n,n];
    wqk8:[2,128,2*h*128] fp8 (q|k per head, x WS); wkq08:[2,128,256] fp8
    (head-0 k|q); wvt:[cc,128,dim]; wot:[h,DH,dim]; y:[nb,n,dim] f16."""
    nc = tc.nc
    Exp = mybir.ActivationFunctionType.Exp
    cc_n = dim // 128  # contraction chunks of the input dim
    jt_n = n // 128  # key tiles per sequence
    it_n = n // 128  # output row tiles per sequence
    i_tot = nb * n  # tokens handled by this core
    nblk = i_tot // 512  # qk-projection column blocks
    dv = DH + 1  # v plus the ones column
    ev = h_ * DH  # total v width across heads

    with ExitStack() as ctx:

        def pool(name, bufs):
            return ctx.enter_context(tc.tile_pool(name=name, bufs=bufs))

        xt_p = pool("xt", cc_n)
        xt8_p = pool("xt8", 2)
        wqk8_p = pool("wqk8", 2)
        wkq08_p = pool("wkq08", 2)
        wvt_p = pool("wvt", cc_n)
        wot_p = pool("wot", h_ // 2)
        v_p = pool("v", i_tot // 128)
        qk_p = pool("qk", 3)
        kq_p = pool("kq", 3)
        e_p = pool("e", jt_n)
        p_p = pool("p", jt_n // 2 + 2)
        eq_p = pool("eq", 4)
        o_p = pool("o", 10)
        raw_p = pool("raw", 4)
        zsp_p = pool("zsp", 3)
        zb_p = pool("zb", 3)
        o2_p = pool("o2", nb * h_ // 2)
        y_p = pool("y", 4)
        psA = ctx.enter_context(
            tc.tile_pool(name="psA", bufs=2, space=bass.MemorySpace.PSUM)
        )
        # dedicated bank pair for the projections (and V/y/broadcast use):
        # keeps the qk projection off the scores-psum rotation, whose tiles
        # recycle only as fast as ACT drains exps.
        psP = ctx.enter_context(
            tc.tile_pool(name="psP", bufs=2, space=bass.MemorySpace.PSUM)
        )
        psV = ctx.enter_context(
            tc.tile_pool(name="psV", bufs=1, space=bass.MemorySpace.PSUM)
        )

        # ---- persistent loads ----
        # Startup is DMA-latency-bound: split the first-needed tensors into
        # pieces spread over all five engine queues (they are all idle at
        # t=0) so the first V-projection matmul can start in ~3us instead
        # of ~11us.
        # two queues only: DMA issues on the scalar queue would block the
        # ACT engine (which must start the v evacuations by ~+8us).
        qs = [nc.sync, nc.gpsimd]
        xt_sb = [
            xt_p.tile([128, i_tot], F16, tag="xt", name=f"xt{c}") for c in range(cc_n)
        ]
        wvt_sb = [
            wvt_p.tile([128, ev], F16, tag="wvt", name=f"wvt{c}") for c in range(cc_n)
        ]
        wot_sb = [
            wot_p.tile([128, dim], F16, tag="wot", name=f"wot{p}")
            for p in range(h_ // 2)
        ]
        # Startup loads round-robined over the three DMA-capable queues in
        # CONSUMPTION order: q/k weights + xt8 feed head-0's projections
        # (the startup critical path), then fp16 xt quarters + wvt for the
        # V projection, then head-0 bias tiles.
        xt8_sb = [
            xt8_p.tile([128, 2 * i_tot], F8, tag="xt8", name=f"xt8_{c}")
            for c in range(2)
        ]
        wqk8_sb = [
            wqk8_p.tile([128, 2 * h_ * 128], F8, tag="wqk8", name=f"wqk8_{c}")
            for c in range(2)
        ]
        wkq08_sb = [
            wkq08_p.tile([128, 256], F8, tag="wkq08", name=f"wkq08_{c}")
            for c in range(2)
        ]
        loads = []
        e_sb0 = []
        for cc2 in range(2):
            loads.append((wqk8_sb[cc2][:], wqk8[cc2, :, :]))
            loads.append((wkq08_sb[cc2][:], wkq08[cc2, :, :]))
        # xt8 split by COLUMN halves (both slots per piece — the projection
        # contracts over both slots per column block)
        hw8 = i_tot // 2
        for hf in range(2):
            for cc2 in range(2):
                dstv = xt8_sb[cc2][:].rearrange("p (s i) -> p s i", s=2)
                srcv = xt8[cc2, :, :].rearrange("p (s i) -> p s i", s=2)
                loads.append(
                    (
                        dstv[:, :, hf * hw8 : (hf + 1) * hw8],
                        srcv[:, :, hf * hw8 : (hf + 1) * hw8],
                    )
                )

        def eb0_load(jp):
            et = e_p.tile([128, 2 * n], F16, tag="e", name=f"e0_{jp}")
            src = eb[0, jp * 256 : (jp + 1) * 256, :].rearrange(
                "(t p) i -> p t i", t=2
            )
            loads.append((et[:].rearrange("p (t i) -> p t i", t=2), src))
            e_sb0.append(et)

        eb0_load(0)
        eb0_load(1)
        qtr = i_tot // 4
        for q4 in range(4):
            sl = slice(q4 * qtr, (q4 + 1) * qtr)
            for cc in range(cc_n):
                loads.append(
                    (xt_sb[cc][:, sl], xt[cc * 128 : (cc + 1) * 128, sl])
                )
            if q4 == 0:
                eb0_load(2)
                eb0_load(3)
            if q4 == 1:
                for cc in range(cc_n):
                    loads.append((wvt_sb[cc][:], wvt[cc, :, :]))
        for hp in range(h_ // 2):
            loads.append((wot_sb[hp][:], wot[2 * hp : 2 * hp + 2, :, :]))
        # three queues at startup: the scalar queue is idle until the first
        # v evacuations (~+12us), so it can carry early pieces too.
        for i, (dst, src) in enumerate(loads):
            (qs + [nc.scalar])[i % 3].dma_start(dst, src)

        ones64 = zsp_p.tile([1, DH], F16, tag="ones")
        nc.gpsimd.memset(ones64[:], 1.0)

        # ---- V projection (all heads), with ones column ----
        # The v evacuation runs on ACT (idle until the first scores land);
        # keeps DVE free for the per-step EB multiplies later. Emitted via a
        # function so head 0's q/k projection chain (which gates the first
        # scores) can be emitted ahead of it.
        v_sb = []

        def v_proj_all():
            for jtg in range(i_tot // 128):
                vt = v_p.tile([128, h_ * dv], F16, tag="v")
                vt3 = vt[:].rearrange("p (h e) -> p h e", h=h_)
                ps = psP.tile([128, ev], F32, tag="psP")
                for cc in range(cc_n):
                    nc.tensor.matmul(
                        ps[:],
                        xt_sb[cc][:, jtg * 128 : (jtg + 1) * 128],
                        wvt_sb[cc][:],
                        start=(cc == 0),
                        stop=(cc == cc_n - 1),
                    )
                nc.gpsimd.memset(vt3[:, :, DH : DH + 1], 1.0)
                nc.scalar.copy(
                    vt3[:, :, 0:DH], ps[:].rearrange("p (h e) -> p h e", h=h_)
                )
                v_sb.append(vt)

        # ---- per-head attention, software-pipelined: head h+1's bias
        # loads, q/k projection and swap are emitted before head h's
        # attention so the PE never waits on the evac+swap chain.
        def proj_pass(hh, nn, qk_t, kq_t=None):
            # one 512-column block of the q/k projection (fp8 DoubleRow,
            # K=256 per pass -> 2 passes) + its DVE evacuation. kq_t given:
            # also project the k|q order (head 0 startup path).
            x8v = [
                xt8_sb[c][:].rearrange("p (s i) -> p s i", s=2) for c in range(2)
            ]
            w8v = [
                wqk8_sb[c][:].rearrange("p (s m) -> p s m", s=2) for c in range(2)
            ]
            ps = psP.tile([128, 512], F32, tag="psP")
            for cc2 in range(2):
                nc.tensor.matmul(
                    ps[:],
                    w8v[cc2][:, :, hh * 128 : (hh + 1) * 128],
                    x8v[cc2][:, :, nn * 512 : (nn + 1) * 512],
                    start=(cc2 == 0),
                    stop=(cc2 == 1),
                    perf_mode=DR,
                )
            nc.vector.tensor_copy(qk_t[:, nn * 512 : (nn + 1) * 512], ps[:])
            if kq_t is not None:
                kq8v = [
                    wkq08_sb[c][:].rearrange("p (s m) -> p s m", s=2)
                    for c in range(2)
                ]
                ps = psP.tile([128, 512], F32, tag="psP")
                for cc2 in range(2):
                    nc.tensor.matmul(
                        ps[:],
                        kq8v[cc2],
                        x8v[cc2][:, :, nn * 512 : (nn + 1) * 512],
                        start=(cc2 == 0),
                        stop=(cc2 == 1),
                        perf_mode=DR,
                    )
                nc.vector.tensor_copy(kq_t[:, nn * 512 : (nn + 1) * 512], ps[:])

        def swap_kq(qk_t, kq_t):
            # partition-swapped copy, split into halves on two queues so the
            # first half-tile lands sooner.
            hw2 = i_tot // 2
            nc.gpsimd.dma_start(kq_t[0:64, 0:hw2], qk_t[64:128, 0:hw2])
            nc.sync.dma_start(kq_t[0:64, hw2:i_tot], qk_t[64:128, hw2:i_tot])
            nc.gpsimd.dma_start(kq_t[64:128, 0:hw2], qk_t[0:64, 0:hw2])
            nc.sync.dma_start(kq_t[64:128, hw2:i_tot], qk_t[0:64, hw2:i_tot])

        def head_prep(hh, e_sb=None, defer=False):
            # exp-bias tiles as j-tile PAIRS: [128, 2*n] = [jA-row | jB-row]
            if e_sb is None:
                e_sb = []
                for jp in range(jt_n // 2):
                    et = e_p.tile([128, 2 * n], F16, tag="e")
                    src = eb[hh, jp * 256 : (jp + 1) * 256, :].rearrange(
                        "(t p) i -> p t i", t=2
                    )
                    eng = nc.sync if jp % 2 == 0 else nc.gpsimd
                    eng.dma_start(et[:].rearrange("p (t i) -> p t i", t=2), src)
                    e_sb.append(et)
            qk_t = qk_p.tile([128, i_tot], F16, tag="qk")
            kq_t = kq_p.tile([128, i_tot], F16, tag="kq")
            if not defer:
                for nn in range(nblk):
                    proj_pass(hh, nn, qk_t, kq_t if hh == 0 else None)
                if hh != 0:
                    swap_kq(qk_t, kq_t)
            return e_sb, qk_t, kq_t

        # Normalize is pipelined 2 steps behind AV: step n copies PSUM out
        # (releasing the accumulator) and spreads Z by DMA; step n+1 runs the
        # reciprocal and launches the broadcast DMA chain; step n+2 does the
        # multiply (broadcast long complete -> no DVE stall).
        o_tiles = {}
        stage1 = []  # (key, raw, zsp)
        stage2 = []  # (key, raw, zb)

        def norm_stage1(fast=False, idx=0):
            key, raw, zsp = stage1.pop(idx)
            if fast:
                # tail path: spread-reciprocal (reciprocal costs ~25
                # cycles/elem, so it must run on the [128, n/128] spread),
                # gather via the idle-at-drain scalar queue, broadcast with
                # K=1 ones-matmuls.
                with nc.allow_low_precision("softmax denominator in fp16"):
                    nc.vector.reciprocal(zsp[:], zsp[:])
                zi = zb_p.tile([1, n], F16, tag="zb")
                nc.scalar.dma_start(zi[:], zsp[:])
                zb = []
                for k in range(n // 512):
                    zt = psP.tile([DH, 512], F32, tag="psP")
                    nc.tensor.matmul(
                        zt[:], ones64[:], zi[0:1, k * 512 : (k + 1) * 512]
                    )
                    zb.append(zt)
            else:
                with nc.allow_low_precision("softmax denominator in fp16"):
                    nc.vector.reciprocal(zsp[:], zsp[:])
                zb = zb_p.tile([DH, n], F16, tag="zb")
                nc.gpsimd.dma_start(zb[0:1, :], zsp[:])
                w = 1
                while w < DH:
                    nc.gpsimd.dma_start(zb[w : 2 * w, :], zb[0:w, :])
                    w *= 2
            stage2.append((key, raw, zb))

        o2 = {}

        def norm_stage2(idx=0):
            key, raw, zb = stage2.pop(idx)
            ot = o_p.tile([DH, n], F16, tag="o")
            if isinstance(zb, list):
                for k, zt in enumerate(zb):
                    sl = slice(k * 512, (k + 1) * 512)
                    nc.vector.tensor_mul(ot[:, sl], raw[0:DH, sl], zt[:])
            else:
                nc.vector.tensor_mul(ot[:], raw[0:DH, :], zb[:])
            o_tiles[key] = ot
            # repack head pairs [128, n] for the K=128 out-projection as
            # soon as both halves exist, so the DMAs overlap attention.
            b, hh = key
            if (b, hh ^ 1) in o_tiles:
                t = o2_p.tile([128, n], F16, tag="o2")
                eng = nc.gpsimd if hh >= h_ - 2 else nc.sync
                eng.dma_start(t[0:64, :], o_tiles[(b, hh & ~1)][:])
                eng.dma_start(t[64:128, :], o_tiles[(b, hh | 1)][:])
                o2[(b, hh // 2)] = t

        # AV for step n is emitted interleaved into step n+1's scores so the
        # PE has AV work to do while ACT catches up on exp evacuations.
        pending = []  # [b, hh, p_sb, pso, jt_next]

        def av_chunk(njt):
            b, hh, p_sb, pso, jt0 = pending[0]
            for jt in range(jt0, min(jt0 + njt, jt_n)):
                poff = (jt & 1) * n
                for ib in range(n // 512):
                    nc.tensor.matmul(
                        pso[:, ib * 512 : (ib + 1) * 512],
                        v_sb[b * jt_n + jt][:, hh * dv : (hh + 1) * dv],
                        p_sb[jt // 2][:, poff + ib * 512 : poff + (ib + 1) * 512],
                        start=(jt == 0),
                        stop=(jt == jt_n - 1),
                    )
            pending[0][4] = min(jt0 + njt, jt_n)
            if pending[0][4] == jt_n:
                b, hh, p_sb, pso, _ = pending.pop(0)
                raw = raw_p.tile([dv, n], F16, tag="raw")
                nc.vector.tensor_copy(raw[:], pso[:])
                zsp = zsp_p.tile([128, n // 128], F16, tag="zsp")
                # last head: the sync queue is clogged with y/output DMAs,
                # the scalar queue is idle once the exps wind down.
                eng = nc.scalar if hh == h_ - 1 else nc.sync
                eng.dma_start(zsp[:], raw[64:65, :])
                stage1.append(((b, hh), raw, zsp))

        def flush_norm(b0):
            # process every outstanding stage belonging to batch b0
            i = 0
            while i < len(stage1):
                if stage1[i][0][0] == b0:
                    norm_stage1(fast=True, idx=i)
                else:
                    i += 1
            i = 0
            while i < len(stage2):
                if stage2[i][0][0] == b0:
                    norm_stage2(idx=i)
                else:
                    i += 1

        def step(b, hh, e_sb, qk_t, kq_t, units=None):
            units = units or []
            if hh == h_ - 1 and b == b_order[-1]:
                # Last step: flush the first batch's norm stages that are
                # already flushable BEFORE this step's EB multiplies hit the
                # DVE queue (its (b0,h7) stage follows mid-step, after the
                # pending AV drains).
                flush_norm(b_order[0])
            if len(stage1) > 1 or (hh == h_ - 1 and stage1):
                # last head: fast path (the gpsimd doubling chain takes
                # ~16us when the drain queues are busy)
                norm_stage1(fast=(hh == h_ - 1))
            # scores -> exp -> *EB, producing P pairs [128, 2n] fp16, with
            # the previous step's AV matmuls interleaved between jp groups.
            p_sb = []
            for jp in range(jt_n // 2):
                p2 = p_p.tile([128, 2 * n], F16, tag="p")
                p3 = p2[:].rearrange("p (t i) -> p t i", t=2)
                e3 = e_sb[jp][:].rearrange("p (t i) -> p t i", t=2)
                jA, jB = 2 * jp, 2 * jp + 1
                jjA = b * n + jA * 128
                jjB = b * n + jB * 128
                ps0 = psA.tile([128, 1024], F32, tag="psA")
                ps1 = psA.tile([128, 1024], F32, tag="psA")
                pss = [ps0, ps1]
                for ib in range(n // 512):
                    ii = b * n + ib * 512
                    nc.tensor.matmul(
                        pss[ib][:, 0:512],
                        kq_t[0:64, jjA : jjA + 128],
                        qk_t[0:64, ii : ii + 512],
                    )
                for ib in range(n // 512):
                    ii = b * n + ib * 512
                    nc.tensor.matmul(
                        pss[ib][:, 512:1024],
                        qk_t[64:128, jjB : jjB + 128],
                        kq_t[64:128, ii : ii + 512],
                    )
                if pending:
                    av_chunk(2)
                for ib in range(n // 512):
                    eq = eq_p.tile([128, 1024], F16, tag="eq")
                    # logits = psum * SCALE/WS^2 (q/k carry a 2^5 prescale)
                    nc.scalar.activation(
                        eq[:], pss[ib][:], Exp, scale=SCALE / (WS * WS)
                    )
                    sl = slice(ib * 512, ib * 512 + 512)
                    nc.vector.tensor_mul(
                        p3[:, :, sl],
                        eq[:].rearrange("p (t i) -> p t i", t=2),
                        e3[:, :, sl],
                    )
                p_sb.append(p2)
                if units:
                    # one next-head projection block: its PE matmuls slot in
                    # here and its DVE cast interleaves with this jp's EB
                    # multiplies instead of queueing behind all of them.
                    units.pop(0)()
            while units:
                units.pop(0)()
            while pending:
                av_chunk(jt_n)
            if stage2:
                norm_stage2()
            if hh == h_ - 1 and b == b_order[-1]:
                # the (b0, h7) stage became flushable when the pending AV
                # drained above; flush it so o2(b0) is complete for the
                # drain's y0/AV interleave.
                flush_norm(b_order[0])
            pso = psV.tile([dv, n], F32, tag="psV")
            pending.append([b, hh, p_sb, pso, 0])

        def y_tile(b, it):
            # psum evac on DVE (ACT may still be running exps of the last
            # head; a Copy there would thrash the activation table), fp16
            # output tile, out-DMA alternating queues.
            psy = psP.tile([128, dim], F32, tag="psP")
            for hp in range(h_ // 2):
                nc.tensor.matmul(
                    psy[:],
                    o2[(b, hp)][:, it * 128 : (it + 1) * 128],
                    wot_sb[hp][:],
                    start=(hp == 0),
                    stop=(hp == h_ // 2 - 1),
                )
            yt = y_p.tile([128, dim], F16, tag="y")
            nc.vector.tensor_copy(yt[:], psy[:])
            eng = nc.sync if it % 2 == 0 else nc.gpsimd
            eng.dma_start(y[b, it * 128 : (it + 1) * 128, :], yt[:])

        def y_proj(b):
            for it in range(it_n):
                y_tile(b, it)

        # head_prep(hh+1) is emitted BETWEEN the two batch steps of head hh:
        # its DVE evacuation casts then sit in the middle of the head's EB
        # multiplies (instead of behind a full step of them), so qk/kq tiles
        # are long ready when head hh+1's scores start.
        b_order = list(range(nb))
        # head 0's projection + evac chain first (it gates the first scores
        # at ~+13us); the V projection fills the PE behind it.
        prep = head_prep(0, e_sb0)
        v_proj_all()
        for hh in range(h_):
            e_sb, qk_t, kq_t = prep
            units = []
            if hh + 1 < h_:
                prep = head_prep(hh + 1, defer=True)
                _, nqk, nkq = prep
                units = [
                    (lambda h2=hh + 1, nn=nn, t=nqk: proj_pass(h2, nn, t))
                    for nn in range(nblk)
                ]
                units.append(lambda t=nqk, u=nkq: swap_kq(t, u))
            for bi, b in enumerate(b_order):
                step(b, hh, e_sb, qk_t, kq_t, units if bi == 0 else None)

        # Drain: finish the last AV first (the PE queue is in-order — y
        # tiles must not block it), then the first batch's y projection
        # (its o2 tiles completed during the last step's flush), then the
        # final normalize and the last batch's y projection.
        while pending:
            av_chunk(jt_n)
        y_proj(b_order[0])
        while stage1:
            norm_stage1(fast=True)
        while stage2:
            norm_stage2()
        for b in b_order[1:]:
            y_proj(b)


def build(nb=NB, h_=H, n=N, dim=DIM):
    nc = bacc.Bacc("TRN2", target_bir_lowering=False, debug=False)
    cc_n = dim // 128
    i_tot = nb * n
    xt_d = nc.dram_tensor("xt", [dim, i_tot], F16, kind="ExternalInput")
    xt8_d = nc.dram_tensor("xt8", [2, 128, 2 * i_tot], F8, kind="ExternalInput")
    eb_d = nc.dram_tensor("eb", [h_, n, n], F16, kind="ExternalInput")
    wqk8_d = nc.dram_tensor("wqk8", [2, 128, 2 * h_ * 128], F8, kind="ExternalInput")
    wkq08_d = nc.dram_tensor("wkq08", [2, 128, 256], F8, kind="ExternalInput")
    wvt_d = nc.dram_tensor("wvt", [cc_n, 128, h_ * DH], F16, kind="ExternalInput")
    wot_d = nc.dram_tensor("wot", [h_, DH, dim], F16, kind="ExternalInput")
    y_d = nc.dram_tensor("y", [nb, n, dim], F16, kind="ExternalOutput")
    with tile.TileContext(nc) as tc:
        emit(
            tc,
            xt_d.ap(),
            xt8_d.ap(),
            eb_d.ap(),
            wqk8_d.ap(),
            wkq08_d.ap(),
            wvt_d.ap(),
            wot_d.ap(),
            y_d.ap(),
            nb=nb,
            h_=h_,
            n=n,
            dim=dim,
        )
    nc.compile()
    return nc


def prep_inputs(x, pos_bias, Wq, Wk, Wv, Wo, nb=NB, h_=H, n=N, dim=DIM):
    """Host-side layout prep. Returns per-core input maps."""
    x = np.asarray(x, np.float32)
    pos_bias = np.asarray(pos_bias, np.float32)
    b_tot = x.shape[0]
    ncores = b_tot // nb
    cc_n = dim // 128
    dh = DH

    xT32 = np.ascontiguousarray(x.reshape(b_tot * n, dim).T)
    xT = xT32.astype(np.float16)
    ebt = np.ascontiguousarray(np.exp(pos_bias).transpose(0, 2, 1)).astype(np.float16)

    # fp8 DoubleRow operands for the q/k projection: channel c maps to
    # (cc2, slot, p) = (c // 256, (c // 128) % 2, c % 128); weights carry a
    # 2^5 prescale to land in e4m3's normal range (compensated in the exp
    # scale), and 1/sqrt(dh) moves to the exp scale as well.
    def dr_layout(a):  # [512, w] -> [2, 128, 2, w]
        return np.ascontiguousarray(a.reshape(2, 2, 128, -1).transpose(0, 2, 1, 3))

    xT8 = dr_layout(xT32).astype(E4)
    wqs = np.asarray(Wq, np.float32).T * WS  # [c, e]
    wkt = np.asarray(Wk, np.float32).T * WS
    wq8 = dr_layout(wqs)  # [2, 128, 2, 512] fp32
    wk8 = dr_layout(wkt)
    wqk8 = np.empty([2, 128, 2, h_ * 128], np.float32)
    wkq08 = np.empty([2, 128, 2, 128], np.float32)
    for hh in range(h_):
        es = slice(hh * dh, (hh + 1) * dh)
        wqk8[:, :, :, hh * 128 : hh * 128 + dh] = wq8[:, :, :, es]
        wqk8[:, :, :, hh * 128 + dh : hh * 128 + 2 * dh] = wk8[:, :, :, es]
    wkq08[:, :, :, 0:dh] = wk8[:, :, :, 0:dh]
    wkq08[:, :, :, dh : 2 * dh] = wq8[:, :, :, 0:dh]
    wqk8 = wqk8.reshape(2, 128, 2 * h_ * 128).astype(E4)
    wkq08 = wkq08.reshape(2, 128, 256).astype(E4)
    wvt = np.ascontiguousarray(np.asarray(Wv, np.float32).T).astype(np.float16)
    wvt = wvt.reshape(cc_n, 128, h_ * dh)
    wot = np.ascontiguousarray(np.asarray(Wo, np.float32).T).astype(np.float16)
    wot = wot.reshape(h_, dh, dim)

    i_tot = nb * n
    in_maps = []
    for c in range(ncores):
        in_maps.append(
            {
                "xt": np.ascontiguousarray(xT[:, c * i_tot : (c + 1) * i_tot]),
                "xt8": np.ascontiguousarray(
                    xT8[:, :, :, c * i_tot : (c + 1) * i_tot]
                ).reshape(2, 128, 2 * i_tot),
                "eb": ebt,
                "wqk8": wqk8,
                "wkq08": wkq08,
                "wvt": wvt,
                "wot": wot,
            }
        )
    return in_maps


def get_built():
    with _lock:
        if "nc" not in _built:
            _built["nc"] = build()
        return _built["nc"]


def run_on_device(in_maps, **kwargs):
    nc = get_built()
    return run_bass_kernel_spmd(nc, in_maps, core_ids=list(range(len(in_maps))), **kwargs)


def kernel(x, pos_bias, Wq, Wk, Wv, Wo):
    in_maps = prep_inputs(x, pos_bias, Wq, Wk, Wv, Wo)
    res = run_on_device(in_maps)
    y = np.concatenate([r["y"] for r in res.results], axis=0)
    return np.ascontiguousarray(y.astype(np.float32))



# revision 77
# speedup vs baseline: 1.0274x; 1.0274x over previous
"""Trainium2 Bass kernel for nn_Attention_13718125543518.

Dense MHA (B=16, N=1024, DIM=512, H=8, DH=64) with additive positional
bias and softmax:  y = softmax(q k^T / sqrt(dh) + pos_bias) v @ Wo^T.

Sharding: data-parallel over batch. Each of the 8 cores processes 2
batches and all 8 heads; no cross-core communication.

Device-side algorithm (per core, all matmul operands fp16, PSUM fp32):
  - host precomputes xT[c,i] (x transposed), EB[h,j,i] = exp(pos_bias[h,i,j]),
    and weight layouts; the 1/sqrt(dh) scale is folded into Wq.
  - qkT_h[:,i] = [Wq_h^T | Wk_h^T]^T . xT  -> [128, 2048] (rows 0:64 = q^T,
    rows 64:128 = k^T), plus a partition-swapped copy kqT (DMA) so both the
    (0,0) and (64,0) PE row-tiles can compute score tiles concurrently.
  - scores s^T[j,i] = sum_d k[j,d] q[i,d] (K=64 matmuls, two per PE pass via
    row tiling), ACT evacuates PSUM with exp(), DVE multiplies by EB
    (factorized softmax: exp(qk+b) = exp(qk)*exp(b); logits are O(6) so no
    max-subtraction is needed).
  - AV: out^T[d,i] = sum_j v'[j,d] P[j,i] with v' = [v_h | 1] (M=65): row 64
    accumulates the softmax denominator Z[i] for free.
  - normalize (pipelined two steps behind AV): spread Z across partitions by
    DMA, exact DVE reciprocal on [128, n/128], broadcast to 64 partitions
    with doubling DMAs on the gpsimd queue, multiply.
  - y[i,f] = sum_h out_h^T . Wo_h^T with head pairs repacked to K=128.
"""

import threading
from contextlib import ExitStack

import ml_dtypes
import numpy as np

import concourse.bacc as bacc
import concourse.bass as bass
import concourse.mybir as mybir
import concourse.tile as tile
from concourse.bass_utils import run_bass_kernel_spmd

B, N, DIM, H, DH = 16, 1024, 512, 8, 64
SCALE = DH**-0.5
NCORES = 8
NB = B // NCORES  # batches per core
F16 = mybir.dt.float16
F32 = mybir.dt.float32
F8 = mybir.dt.float8e4
E4 = ml_dtypes.float8_e4m3
DR = mybir.MatmulPerfMode.DoubleRow
# q/k weights are pre-scaled by 2^5 into e4m3's normal range (their raw
# sigma 0.02 sits in the subnormals); the 1/WS^2 and the 1/sqrt(dh) of the
# attention logits are folded into the exp's scale parameter.
WS = 32.0

_lock = threading.Lock()
_built = {}


def emit(tc, xt, xt8, eb, wqk8, wkq08, wvt, wot, y, nb=NB, h_=H, n=N, dim=DIM):
    """Emit the per-core program. xt:[dim,nb*n] f16; xt8:[2,128,2*nb*n] fp8
    (DoubleRow layout: channel c = cc2*256 + slot*128 + p); eb:[h,n,n];
    wqk8:[2,128,2*h*128] fp8 (q|k per head, x WS); wkq08:[2,128,256] fp8
    (head-0 k|q); wvt:[cc,128,dim]; wot:[h,DH,dim]; y:[nb,n,dim] f16."""
    nc = tc.nc
    Exp = mybir.ActivationFunctionType.Exp
    cc_n = dim // 128  # contraction chunks of the input dim
    jt_n = n // 128  # key tiles per sequence
    it_n = n // 128  # output row tiles per sequence
    i_tot = nb * n  # tokens handled by this core
    nblk = i_tot // 512  # qk-projection column blocks
    dv = DH + 1  # v plus the ones column
    ev = h_ * DH  # total v width across heads

    with ExitStack() as ctx:

        def pool(name, bufs):
            return ctx.enter_context(tc.tile_pool(name=name, bufs=bufs))

        xt_p = pool("xt", cc_n)
        xt8_p = pool("xt8", 2)
        wqk8_p = pool("wqk8", 2)
        wkq08_p = pool("wkq08", 2)
        wvt_p = pool("wvt", cc_n)
        wot_p = pool("wot", h_ // 2)
        v_p = pool("v", i_tot // 128)
        qk_p = pool("qk", 3)
        kq_p = pool("kq", 3)
        e_p = pool("e", jt_n)
        p_p = pool("p", jt_n // 2 + 2)
        eq_p = pool("eq", 4)
        o_p = pool("o", 10)
        raw_p = pool("raw", 4)
        zsp_p = pool("zsp", 3)
        zb_p = pool("zb", 3)
        o2_p = pool("o2", nb * h_ // 2)
        y_p = pool("y", 4)
        psA = ctx.enter_context(
            tc.tile_pool(name="psA", bufs=2, space=bass.MemorySpace.PSUM)
        )
        # dedicated bank pair for the projections (and V/y/broadcast use):
        # keeps the qk projection off the scores-psum rotation, whose tiles
        # recycle only as fast as ACT drains exps.
        psP = ctx.enter_context(
            tc.tile_pool(name="psP", bufs=2, space=bass.MemorySpace.PSUM)
        )
        psV = ctx.enter_context(
            tc.tile_pool(name="psV", bufs=1, space=bass.MemorySpace.PSUM)
        )

        # ---- persistent loads ----
        # Startup is DMA-latency-bound: split the first-needed tensors into
        # pieces spread over all five engine queues (they are all idle at
        # t=0) so the first V-projection matmul can start in ~3us instead
        # of ~11us.
        # two queues only: DMA issues on the scalar queue would block the
        # ACT engine (which must start the v evacuations by ~+8us).
        qs = [nc.sync, nc.gpsimd]
        xt_sb = [
            xt_p.tile([128, i_tot], F16, tag="xt", name=f"xt{c}") for c in range(cc_n)
        ]
        wvt_sb = [
            wvt_p.tile([128, ev], F16, tag="wvt", name=f"wvt{c}") for c in range(cc_n)
        ]
        wot_sb = [
            wot_p.tile([128, dim], F16, tag="wot", name=f"wot{p}")
            for p in range(h_ // 2)
        ]
        # Startup loads round-robined over the three DMA-capable queues in
        # CONSUMPTION order: q/k weights + xt8 feed head-0's projections
        # (the startup critical path), then fp16 xt quarters + wvt for the
        # V projection, then head-0 bias tiles.
        xt8_sb = [
            xt8_p.tile([128, 2 * i_tot], F8, tag="xt8", name=f"xt8_{c}")
            for c in range(2)
        ]
        wqk8_sb = [
            wqk8_p.tile([128, 2 * h_ * 128], F8, tag="wqk8", name=f"wqk8_{c}")
            for c in range(2)
        ]
        wkq08_sb = [
            wkq08_p.tile([128, 256], F8, tag="wkq08", name=f"wkq08_{c}")
            for c in range(2)
        ]
        loads = []
        e_sb0 = []
        for cc2 in range(2):
            loads.append((wqk8_sb[cc2][:], wqk8[cc2, :, :]))
            loads.append((wkq08_sb[cc2][:], wkq08[cc2, :, :]))
        # xt8 split by COLUMN halves (both slots per piece — the projection
        # contracts over both slots per column block)
        hw8 = i_tot // 2
        for hf in range(2):
            for cc2 in range(2):
                dstv = xt8_sb[cc2][:].rearrange("p (s i) -> p s i", s=2)
                srcv = xt8[cc2, :, :].rearrange("p (s i) -> p s i", s=2)
                loads.append(
                    (
                        dstv[:, :, hf * hw8 : (hf + 1) * hw8],
                        srcv[:, :, hf * hw8 : (hf + 1) * hw8],
                    )
                )

        def eb0_load(jp):
            et = e_p.tile([128, 2 * n], F16, tag="e", name=f"e0_{jp}")
            src = eb[0, jp * 256 : (jp + 1) * 256, :].rearrange(
                "(t p) i -> p t i", t=2
            )
            loads.append((et[:].rearrange("p (t i) -> p t i", t=2), src))
            e_sb0.append(et)

        eb0_load(0)
        eb0_load(1)
        qtr = i_tot // 4
        for q4 in range(4):
            sl = slice(q4 * qtr, (q4 + 1) * qtr)
            for cc in range(cc_n):
                loads.append(
                    (xt_sb[cc][:, sl], xt[cc * 128 : (cc + 1) * 128, sl])
                )
            if q4 == 0:
                eb0_load(2)
                eb0_load(3)
            if q4 == 1:
                for cc in range(cc_n):
                    loads.append((wvt_sb[cc][:], wvt[cc, :, :]))
        for hp in range(h_ // 2):
            loads.append((wot_sb[hp][:], wot[2 * hp : 2 * hp + 2, :, :]))
        # three queues at startup: the scalar queue is idle until the first
        # v evacuations (~+12us), so it can carry early pieces too.
        for i, (dst, src) in enumerate(loads):
            (qs + [nc.scalar])[i % 3].dma_start(dst, src)

        ones64 = zsp_p.tile([1, DH], F16, tag="ones")
        nc.gpsimd.memset(ones64[:], 1.0)

        # ---- V projection (all heads), with ones column ----
        # Emitted as per-j-tile units interleaved into head 0's steps: the
        # v evacuations (ACT) then slot BETWEEN the first exps instead of
        # blocking them on the in-order ACT queue.
        v_sb = [
            v_p.tile([128, h_ * dv], F16, tag="v", name=f"v{j}")
            for j in range(i_tot // 128)
        ]

        def v_unit(jtg):
            vt = v_sb[jtg]
            vt3 = vt[:].rearrange("p (h e) -> p h e", h=h_)
            ps = psP.tile([128, ev], F32, tag="psP")
            for cc in range(cc_n):
                nc.tensor.matmul(
                    ps[:],
                    xt_sb[cc][:, jtg * 128 : (jtg + 1) * 128],
                    wvt_sb[cc][:],
                    start=(cc == 0),
                    stop=(cc == cc_n - 1),
                )
            nc.gpsimd.memset(vt3[:, :, DH : DH + 1], 1.0)
            nc.scalar.copy(
                vt3[:, :, 0:DH], ps[:].rearrange("p (h e) -> p h e", h=h_)
            )

        # ---- per-head attention, software-pipelined: head h+1's bias
        # loads, q/k projection and swap are emitted before head h's
        # attention so the PE never waits on the evac+swap chain.
        def proj_pass(hh, nn, qk_t, kq_t=None):
            # one 512-column block of the q/k projection (fp8 DoubleRow,
            # K=256 per pass -> 2 passes) + its DVE evacuation. kq_t given:
            # also project the k|q order (head 0 startup path).
            x8v = [
                xt8_sb[c][:].rearrange("p (s i) -> p s i", s=2) for c in range(2)
            ]
            w8v = [
                wqk8_sb[c][:].rearrange("p (s m) -> p s m", s=2) for c in range(2)
            ]
            ps = psP.tile([128, 512], F32, tag="psP")
            for cc2 in range(2):
                nc.tensor.matmul(
                    ps[:],
                    w8v[cc2][:, :, hh * 128 : (hh + 1) * 128],
                    x8v[cc2][:, :, nn * 512 : (nn + 1) * 512],
                    start=(cc2 == 0),
                    stop=(cc2 == 1),
                    perf_mode=DR,
                )
            nc.vector.tensor_copy(qk_t[:, nn * 512 : (nn + 1) * 512], ps[:])
            if kq_t is not None:
                kq8v = [
                    wkq08_sb[c][:].rearrange("p (s m) -> p s m", s=2)
                    for c in range(2)
                ]
                ps = psP.tile([128, 512], F32, tag="psP")
                for cc2 in range(2):
                    nc.tensor.matmul(
                        ps[:],
                        kq8v[cc2],
                        x8v[cc2][:, :, nn * 512 : (nn + 1) * 512],
                        start=(cc2 == 0),
                        stop=(cc2 == 1),
                        perf_mode=DR,
                    )
                nc.vector.tensor_copy(kq_t[:, nn * 512 : (nn + 1) * 512], ps[:])

        def swap_kq(qk_t, kq_t):
            # partition-swapped copy, split into halves on two queues so the
            # first half-tile lands sooner.
            hw2 = i_tot // 2
            nc.gpsimd.dma_start(kq_t[0:64, 0:hw2], qk_t[64:128, 0:hw2])
            nc.sync.dma_start(kq_t[0:64, hw2:i_tot], qk_t[64:128, hw2:i_tot])
            nc.gpsimd.dma_start(kq_t[64:128, 0:hw2], qk_t[0:64, 0:hw2])
            nc.sync.dma_start(kq_t[64:128, hw2:i_tot], qk_t[0:64, hw2:i_tot])

        def head_prep(hh, e_sb=None, defer=False):
            # exp-bias tiles as j-tile PAIRS: [128, 2*n] = [jA-row | jB-row]
            if e_sb is None:
                e_sb = []
                for jp in range(jt_n // 2):
                    et = e_p.tile([128, 2 * n], F16, tag="e")
                    src = eb[hh, jp * 256 : (jp + 1) * 256, :].rearrange(
                        "(t p) i -> p t i", t=2
                    )
                    eng = nc.sync if jp % 2 == 0 else nc.gpsimd
                    eng.dma_start(et[:].rearrange("p (t i) -> p t i", t=2), src)
                    e_sb.append(et)
            qk_t = qk_p.tile([128, i_tot], F16, tag="qk")
            kq_t = kq_p.tile([128, i_tot], F16, tag="kq")
            if not defer:
                for nn in range(nblk):
                    proj_pass(hh, nn, qk_t, kq_t if hh == 0 else None)
                if hh != 0:
                    swap_kq(qk_t, kq_t)
            return e_sb, qk_t, kq_t

        # Normalize is pipelined 2 steps behind AV: step n copies PSUM out
        # (releasing the accumulator) and spreads Z by DMA; step n+1 runs the
        # reciprocal and launches the broadcast DMA chain; step n+2 does the
        # multiply (broadcast long complete -> no DVE stall).
        o_tiles = {}
        stage1 = []  # (key, raw, zsp)
        stage2 = []  # (key, raw, zb)

        def norm_stage1(fast=False, idx=0):
            key, raw, zsp = stage1.pop(idx)
            if fast:
                # tail path: spread-reciprocal (reciprocal costs ~25
                # cycles/elem, so it must run on the [128, n/128] spread),
                # gather via the idle-at-drain scalar queue, broadcast with
                # K=1 ones-matmuls.
                with nc.allow_low_precision("softmax denominator in fp16"):
                    nc.vector.reciprocal(zsp[:], zsp[:])
                zi = zb_p.tile([1, n], F16, tag="zb")
                nc.scalar.dma_start(zi[:], zsp[:])
                zb = []
                for k in range(n // 512):
                    zt = psP.tile([DH, 512], F32, tag="psP")
                    nc.tensor.matmul(
                        zt[:], ones64[:], zi[0:1, k * 512 : (k + 1) * 512]
                    )
                    zb.append(zt)
            else:
                with nc.allow_low_precision("softmax denominator in fp16"):
                    nc.vector.reciprocal(zsp[:], zsp[:])
                zb = zb_p.tile([DH, n], F16, tag="zb")
                nc.gpsimd.dma_start(zb[0:1, :], zsp[:])
                w = 1
                while w < DH:
                    nc.gpsimd.dma_start(zb[w : 2 * w, :], zb[0:w, :])
                    w *= 2
            stage2.append((key, raw, zb))

        o2 = {}

        def norm_stage2(idx=0):
            key, raw, zb = stage2.pop(idx)
            ot = o_p.tile([DH, n], F16, tag="o")
            if isinstance(zb, list):
                for k, zt in enumerate(zb):
                    sl = slice(k * 512, (k + 1) * 512)
                    nc.vector.tensor_mul(ot[:, sl], raw[0:DH, sl], zt[:])
            else:
                nc.vector.tensor_mul(ot[:], raw[0:DH, :], zb[:])
            o_tiles[key] = ot
            # repack head pairs [128, n] for the K=128 out-projection as
            # soon as both halves exist, so the DMAs overlap attention.
            b, hh = key
            if (b, hh ^ 1) in o_tiles:
                t = o2_p.tile([128, n], F16, tag="o2")
                eng = nc.gpsimd if hh >= h_ - 2 else nc.sync
                eng.dma_start(t[0:64, :], o_tiles[(b, hh & ~1)][:])
                eng.dma_start(t[64:128, :], o_tiles[(b, hh | 1)][:])
                o2[(b, hh // 2)] = t

        # AV for step n is emitted interleaved into step n+1's scores so the
        # PE has AV work to do while ACT catches up on exp evacuations.
        pending = []  # [b, hh, p_sb, pso, jt_next]

        def av_chunk(njt):
            b, hh, p_sb, pso, jt0 = pending[0]
            for jt in range(jt0, min(jt0 + njt, jt_n)):
                poff = (jt & 1) * n
                for ib in range(n // 512):
                    nc.tensor.matmul(
                        pso[:, ib * 512 : (ib + 1) * 512],
                        v_sb[b * jt_n + jt][:, hh * dv : (hh + 1) * dv],
                        p_sb[jt // 2][:, poff + ib * 512 : poff + (ib + 1) * 512],
                        start=(jt == 0),
                        stop=(jt == jt_n - 1),
                    )
            pending[0][4] = min(jt0 + njt, jt_n)
            if pending[0][4] == jt_n:
                b, hh, p_sb, pso, _ = pending.pop(0)
                raw = raw_p.tile([dv, n], F16, tag="raw")
                nc.vector.tensor_copy(raw[:], pso[:])
                zsp = zsp_p.tile([128, n // 128], F16, tag="zsp")
                # last head: the sync queue is clogged with y/output DMAs,
                # the scalar queue is idle once the exps wind down.
                eng = nc.scalar if hh == h_ - 1 else nc.sync
                eng.dma_start(zsp[:], raw[64:65, :])
                stage1.append(((b, hh), raw, zsp))

        def flush_norm(b0):
            # process every outstanding stage belonging to batch b0
            i = 0
            while i < len(stage1):
                if stage1[i][0][0] == b0:
                    norm_stage1(fast=True, idx=i)
                else:
                    i += 1
            i = 0
            while i < len(stage2):
                if stage2[i][0][0] == b0:
                    norm_stage2(idx=i)
                else:
                    i += 1

        def step(b, hh, e_sb, qk_t, kq_t, units=None):
            units = units or []
            if hh == h_ - 1 and b == b_order[-1]:
                # Last step: flush the first batch's norm stages that are
                # already flushable BEFORE this step's EB multiplies hit the
                # DVE queue (its (b0,h7) stage follows mid-step, after the
                # pending AV drains).
                flush_norm(b_order[0])
            if len(stage1) > 1 or (hh == h_ - 1 and stage1):
                # last head: fast path (the gpsimd doubling chain takes
                # ~16us when the drain queues are busy)
                norm_stage1(fast=(hh == h_ - 1))
            # scores -> exp -> *EB, producing P pairs [128, 2n] fp16, with
            # the previous step's AV matmuls interleaved between jp groups.
            p_sb = []
            for jp in range(jt_n // 2):
                p2 = p_p.tile([128, 2 * n], F16, tag="p")
                p3 = p2[:].rearrange("p (t i) -> p t i", t=2)
                e3 = e_sb[jp][:].rearrange("p (t i) -> p t i", t=2)
                jA, jB = 2 * jp, 2 * jp + 1
                jjA = b * n + jA * 128
                jjB = b * n + jB * 128
                ps0 = psA.tile([128, 1024], F32, tag="psA")
                ps1 = psA.tile([128, 1024], F32, tag="psA")
                pss = [ps0, ps1]
                for ib in range(n // 512):
                    ii = b * n + ib * 512
                    nc.tensor.matmul(
                        pss[ib][:, 0:512],
                        kq_t[0:64, jjA : jjA + 128],
                        qk_t[0:64, ii : ii + 512],
                    )
                for ib in range(n // 512):
                    ii = b * n + ib * 512
                    nc.tensor.matmul(
                        pss[ib][:, 512:1024],
                        qk_t[64:128, jjB : jjB + 128],
                        kq_t[64:128, ii : ii + 512],
                    )
                if pending:
                    av_chunk(2)
                for ib in range(n // 512):
                    eq = eq_p.tile([128, 1024], F16, tag="eq")
                    # logits = psum * SCALE/WS^2 (q/k carry a 2^5 prescale)
                    nc.scalar.activation(
                        eq[:], pss[ib][:], Exp, scale=SCALE / (WS * WS)
                    )
                    sl = slice(ib * 512, ib * 512 + 512)
                    nc.vector.tensor_mul(
                        p3[:, :, sl],
                        eq[:].rearrange("p (t i) -> p t i", t=2),
                        e3[:, :, sl],
                    )
                p_sb.append(p2)
                if units:
                    # one next-head projection block: its PE matmuls slot in
                    # here and its DVE cast interleaves with this jp's EB
                    # multiplies instead of queueing behind all of them.
                    units.pop(0)()
            while units:
                units.pop(0)()
            while pending:
                av_chunk(jt_n)
            if stage2:
                norm_stage2()
            if hh == h_ - 1 and b == b_order[-1]:
                # the (b0, h7) stage became flushable when the pending AV
                # drained above; flush it so o2(b0) is complete for the
                # drain's y0/AV interleave.
                flush_norm(b_order[0])
            pso = psV.tile([dv, n], F32, tag="psV")
            pending.append([b, hh, p_sb, pso, 0])

        def y_tile(b, it):
            # psum evac on DVE (ACT may still be running exps of the last
            # head; a Copy there would thrash the activation table), fp16
            # output tile, out-DMA alternating queues.
            psy = psP.tile([128, dim], F32, tag="psP")
            for hp in range(h_ // 2):
                nc.tensor.matmul(
                    psy[:],
                    o2[(b, hp)][:, it * 128 : (it + 1) * 128],
                    wot_sb[hp][:],
                    start=(hp == 0),
                    stop=(hp == h_ // 2 - 1),
                )
            yt = y_p.tile([128, dim], F16, tag="y")
            nc.vector.tensor_copy(yt[:], psy[:])
            eng = nc.sync if it % 2 == 0 else nc.gpsimd
            eng.dma_start(y[b, it * 128 : (it + 1) * 128, :], yt[:])

        def y_proj(b):
            for it in range(it_n):
                y_tile(b, it)

        # head_prep(hh+1) is emitted BETWEEN the two batch steps of head hh:
        # its DVE evacuation casts then sit in the middle of the head's EB
        # multiplies (instead of behind a full step of them), so qk/kq tiles
        # are long ready when head hh+1's scores start.
        b_order = list(range(nb))
        # head 0's projection + evac chain first (it gates the first scores
        # at ~+13us).
        prep = head_prep(0, e_sb0)
        for hh in range(h_):
            e_sb, qk_t, kq_t = prep
            punits = []
            if hh + 1 < h_:
                prep = head_prep(hh + 1, defer=True)
                _, nqk, nkq = prep
                punits = [
                    (lambda h2=hh + 1, nn=nn, t=nqk: proj_pass(h2, nn, t))
                    for nn in range(nblk)
                ]
                punits.append(lambda t=nqk, u=nkq: swap_kq(t, u))
            if hh == 0:
                u0 = [(lambda j=j: v_unit(j)) for j in range(jt_n)]
                u1 = [(lambda j=jt_n + j: v_unit(j)) for j in range(jt_n)]
                u1 += punits
            else:
                u0, u1 = punits, []
            step(b_order[0], hh, e_sb, qk_t, kq_t, u0)
            step(b_order[1], hh, e_sb, qk_t, kq_t, u1)

        # Drain: finish the last AV first (the PE queue is in-order — y
        # tiles must not block it), then the first batch's y projection
        # (its o2 tiles completed during the last step's flush), then the
        # final normalize and the last batch's y projection.
        while pending:
            av_chunk(jt_n)
        y_proj(b_order[0])
        while stage1:
            norm_stage1(fast=True)
        while stage2:
            norm_stage2()
        for b in b_order[1:]:
            y_proj(b)


def build(nb=NB, h_=H, n=N, dim=DIM):
    nc = bacc.Bacc("TRN2", target_bir_lowering=False, debug=False)
    cc_n = dim // 128
    i_tot = nb * n
    xt_d = nc.dram_tensor("xt", [dim, i_tot], F16, kind="ExternalInput")
    xt8_d = nc.dram_tensor("xt8", [2, 128, 2 * i_tot], F8, kind="ExternalInput")
    eb_d = nc.dram_tensor("eb", [h_, n, n], F16, kind="ExternalInput")
    wqk8_d = nc.dram_tensor("wqk8", [2, 128, 2 * h_ * 128], F8, kind="ExternalInput")
    wkq08_d = nc.dram_tensor("wkq08", [2, 128, 256], F8, kind="ExternalInput")
    wvt_d = nc.dram_tensor("wvt", [cc_n, 128, h_ * DH], F16, kind="ExternalInput")
    wot_d = nc.dram_tensor("wot", [h_, DH, dim], F16, kind="ExternalInput")
    y_d = nc.dram_tensor("y", [nb, n, dim], F16, kind="ExternalOutput")
    with tile.TileContext(nc) as tc:
        emit(
            tc,
            xt_d.ap(),
            xt8_d.ap(),
            eb_d.ap(),
            wqk8_d.ap(),
            wkq08_d.ap(),
            wvt_d.ap(),
            wot_d.ap(),
            y_d.ap(),
            nb=nb,
            h_=h_,
            n=n,
            dim=dim,
        )
    nc.compile()
    return nc


def prep_inputs(x, pos_bias, Wq, Wk, Wv, Wo, nb=NB, h_=H, n=N, dim=DIM):
    """Host-side layout prep. Returns per-core input maps."""
    x = np.asarray(x, np.float32)
    pos_bias = np.asarray(pos_bias, np.float32)
    b_tot = x.shape[0]
    ncores = b_tot // nb
    cc_n = dim // 128
    dh = DH

    xT32 = np.ascontiguousarray(x.reshape(b_tot * n, dim).T)
    xT = xT32.astype(np.float16)
    ebt = np.ascontiguousarray(np.exp(pos_bias).transpose(0, 2, 1)).astype(np.float16)

    # fp8 DoubleRow operands for the q/k projection: channel c maps to
    # (cc2, slot, p) = (c // 256, (c // 128) % 2, c % 128); weights carry a
    # 2^5 prescale to land in e4m3's normal range (compensated in the exp
    # scale), and 1/sqrt(dh) moves to the exp scale as well.
    def dr_layout(a):  # [512, w] -> [2, 128, 2, w]
        return np.ascontiguousarray(a.reshape(2, 2, 128, -1).transpose(0, 2, 1, 3))

    xT8 = dr_layout(xT32).astype(E4)
    wqs = np.asarray(Wq, np.float32).T * WS  # [c, e]
    wkt = np.asarray(Wk, np.float32).T * WS
    wq8 = dr_layout(wqs)  # [2, 128, 2, 512] fp32
    wk8 = dr_layout(wkt)
    wqk8 = np.empty([2, 128, 2, h_ * 128], np.float32)
    wkq08 = np.empty([2, 128, 2, 128], np.float32)
    for hh in range(h_):
        es = slice(hh * dh, (hh + 1) * dh)
        wqk8[:, :, :, hh * 128 : hh * 128 + dh] = wq8[:, :, :, es]
        wqk8[:, :, :, hh * 128 + dh : hh * 128 + 2 * dh] = wk8[:, :, :, es]
    wkq08[:, :, :, 0:dh] = wk8[:, :, :, 0:dh]
    wkq08[:, :, :, dh : 2 * dh] = wq8[:, :, :, 0:dh]
    wqk8 = wqk8.reshape(2, 128, 2 * h_ * 128).astype(E4)
    wkq08 = wkq08.reshape(2, 128, 256).astype(E4)
    wvt = np.ascontiguousarray(np.asarray(Wv, np.float32).T).astype(np.float16)
    wvt = wvt.reshape(cc_n, 128, h_ * dh)
    wot = np.ascontiguousarray(np.asarray(Wo, np.float32).T).astype(np.float16)
    wot = wot.reshape(h_, dh, dim)

    i_tot = nb * n
    in_maps = []
    for c in range(ncores):
        in_maps.append(
            {
                "xt": np.ascontiguousarray(xT[:, c * i_tot : (c + 1) * i_tot]),
                "xt8": np.ascontiguousarray(
                    xT8[:, :, :, c * i_tot : (c + 1) * i_tot]
                ).reshape(2, 128, 2 * i_tot),
                "eb": ebt,
                "wqk8": wqk8,
                "wkq08": wkq08,
                "wvt": wvt,
                "wot": wot,
            }
        )
    return in_maps


def get_built():
    with _lock:
        if "nc" not in _built:
            _built["nc"] = build()
        return _built["nc"]


def run_on_device(in_maps, **kwargs):
    nc = get_built()
    return run_bass_kernel_spmd(nc, in_maps, core_ids=list(range(len(in_maps))), **kwargs)


def kernel(x, pos_bias, Wq, Wk, Wv, Wo):
    in_maps = prep_inputs(x, pos_bias, Wq, Wk, Wv, Wo)
    res = run_on_device(in_maps)
    y = np.concatenate([r["y"] for r in res.results], axis=0)
    return np.ascontiguousarray(y.astype(np.float32))

